# revision 9
# baseline (speedup 1.0000x reference)
"""Trainium2 Bass kernel for the autoregressive LSTM decoder.

Data-parallel over batch (512 -> 64 per core x 8 cores). The T=128-step
scan runs fully unrolled per core. Key structural choices:

- The per-step decoder-head logits ARE the final outputs (the reference's
  second seq2seq pass recomputes exactly them), so we stream them to DRAM
  from inside the scan and skip the final pass entirely.
- The argmax->embedding feedback is computed as one-hot matmuls against
  host-prefolded matrices (emb @ Wx), so the embedding-sum and the e@Wx
  of the next step collapse into the gates' PSUM accumulation.
- The pos -> e_next feedback path folds entirely into the recurrent
  weight matrix: Wh_fb = Wh + W_pos @ W_pos_in @ Wx.
- Gate columns are permuted (i,f,o,g) so one sigmoid covers [0:768].
- Everything is fp32: argmax margins in this model go down to ~1e-8, so
  any precision loss in the recurrence cascades.
"""

import sys

sys.path.insert(0, "/opt/trn_rl_repo")

import numpy as np

import bass_rust
from concourse import bass, mybir, tile
from concourse import masks
from concourse.alu_op_type import AluOpType
from concourse.bass_utils import run_bass_kernel_spmd
from concourse.vector_clock import ScopedClock

F32 = mybir.dt.float32
AF = mybir.ActivationFunctionType

N_CORES = 8
B = 64          # batch per core
L = 256         # latent
T = 128         # steps
NG = 1024       # 4*L gate width
NH = 804        # heads: 32 type + 768 color + 4 pos
_MAXW = 1       # walrus CTRL-instruction sync-wait limit workaround


def _patch_tile_drain():
    """walrus rejects >N sync-waits on one CTRL instruction; Tile's exit
    drain waits on every live semaphore. Split the waits across nops."""
    if getattr(tile.TileContext, "_drain_patched", False):
        return

    def _drain_and_barrier(self, tick_clock, wait_clock):
        probe = self.nc.sync.nop(nofuse=True)
        wait_clock.add_sem_waits(probe.ins, ScopedClock({None: tick_clock.global_clock}))
        si = probe.ins.sync_info
        waits = list(si.on_wait) if si and si.on_wait else []
        if len(waits) > _MAXW:
            probe.ins.sync_info = bass_rust.SyncInfo(
                on_wait=waits[:_MAXW], on_update=list(si.on_update or []))
            rest = waits[_MAXW:]
            while rest:
                nop = self.nc.sync.nop(nofuse=True)
                nop.ins.sync_info = bass_rust.SyncInfo(on_wait=rest[:_MAXW], on_update=[])
                rest = rest[_MAXW:]
        drain_inst = self.nc.sync.drain()
        wait_clock.add_sem_waits(drain_inst.ins, ScopedClock({None: tick_clock.global_clock}))
        dsi = drain_inst.ins.sync_info
        if dsi and dsi.on_wait and len(dsi.on_wait) > _MAXW:
            drain_inst.ins.sync_info = bass_rust.SyncInfo(
                on_wait=list(dsi.on_wait)[:_MAXW], on_update=list(dsi.on_update or []))
        self.nc.all_engine_barrier()
        assert self.sems is not None
        popped = self.nc._tile_sem_poison_stack.pop()
        assert popped is self._sem_poison
        self.nc.clear_and_free_semaphores(list(self.sems.allocated().values()))
        self.nc.all_engine_barrier()

    tile.TileContext._drain_and_barrier = _drain_and_barrier

    _orig_commit = tile.TileContext._commit_instruction

    def _commit_instruction(self, inst, lazy_reg_writes=True):
        si = getattr(inst, "sync_info", None)
        if si and si.on_wait and len(si.on_wait) > _MAXW:
            eng = self.nc.engines.get(inst.engine) if hasattr(self.nc.engines, "get") else None
            if eng is None:
                try:
                    eng = self.nc.engines[inst.engine]
                except Exception:
                    eng = None
            if eng is not None:
                waits = list(si.on_wait)
                inst.sync_info = bass_rust.SyncInfo(
                    on_wait=waits[-_MAXW:], on_update=list(si.on_update or []))
                extra = waits[:-_MAXW]
                while extra:
                    nop = eng.nop(nofuse=True)
                    nop.ins.sync_info = bass_rust.SyncInfo(on_wait=extra[:_MAXW], on_update=[])
                    extra = extra[_MAXW:]
        return _orig_commit(self, inst, lazy_reg_writes)

    tile.TileContext._commit_instruction = _commit_instruction
    tile.TileContext._drain_patched = True


def fold_weights(inp):
    """Host-side exact weight transforms (fp64 accumulate, fp32 store)."""
    f8 = lambda x: np.asarray(x, np.float64)
    Wx, Wh = f8(inp["Wx"]), f8(inp["Wh"])
    # gate permutation: new order (i, f, o, g)
    P = np.concatenate([np.arange(0, 512), np.arange(768, 1024), np.arange(512, 768)])
    WxP, WhP = Wx[:, P], Wh[:, P]
    A = f8(inp["W_pos_in"]) @ WxP                         # [4, 1024]
    out = {}
    # The kernel carries the cell state as s=2c / h2=2h (sigmoid computed as
    # 0.5 + 0.5*tanh(x/2) with the halvings folded away); weights that
    # multiply h2 absorb an exact 0.5.
    out["Wh_plain"] = WhP.astype(np.float32)
    out["Wh_fb"] = (0.5 * (WhP + f8(inp["W_pos"]) @ A)).astype(np.float32)
    out["Mcolor"] = (f8(inp["emb_color"]) @ WxP).astype(np.float32)
    bias_fb = f8(inp["b_lstm"])[P] + f8(inp["b_pos_in"]) @ WxP + f8(inp["b_pos"]) @ A
    out["Mtype33"] = np.concatenate(
        [f8(inp["emb_type"]) @ WxP, bias_fb[None, :]], 0).astype(np.float32)
    out["bias0"] = (f8(inp["bos"][0, 0]) @ WxP + f8(inp["b_lstm"])[P]).astype(np.float32)[None, :]
    out["W_heads"] = (0.5 * np.concatenate(
        [f8(inp["W_type"]), f8(inp["W_color"]), f8(inp["W_pos"])], 1)).astype(np.float32)
    out["b_heads"] = np.concatenate(
        [inp["b_type"], inp["b_color"], inp["b_pos"]]).astype(np.float32)[None, :]
    out["Wh0"] = np.asarray(inp["Wh0"], np.float32)
    out["Wc0"] = np.asarray(inp["Wc0"], np.float32)
    out["bh0"] = np.asarray(inp["bh0"], np.float32)[None, :]
    out["bc0"] = np.asarray(inp["bc0"], np.float32)[None, :]
    out["W_len"] = np.asarray(inp["W_len"], np.float32)
    out["b_len"] = np.asarray(inp["b_len"], np.float32)[None, :]
    return out


def build_nc(n_steps=T):
    nc = bass.Bass("TRN2")
    d = {}
    def din(name, shape):
        d[name] = nc.declare_dram_parameter(name, list(shape), F32, isOutput=False)
    def dout(name, shape):
        d[name] = nc.declare_dram_parameter(name, list(shape), F32, isOutput=True)

    din("z", (B, L))
    din("Wh_plain", (L, NG)); din("Wh_fb", (L, NG)); din("Mcolor", (L, NG))
    din("Mtype33", (33, NG)); din("bias0", (1, NG))
    din("W_heads", (L, NH)); din("b_heads", (1, NH))
    din("Wh0", (L, L)); din("Wc0", (L, L)); din("bh0", (1, L)); din("bc0", (1, L))
    din("W_len", (L, T)); din("b_len", (1, T))
    dout("out_type", (B, T, 32)); dout("out_pos", (B, T, 4))
    dout("out_color", (B, T, 768)); dout("out_len", (B, T))

    with tile.TileContext(nc) as tc:
        with (
            tc.tile_pool(name="wpool", bufs=1) as wp,
            tc.tile_pool(name="state", bufs=2) as sp,
            tc.tile_pool(name="work", bufs=2) as kp,
            tc.tile_pool(name="stage", bufs=3) as gp,
            tc.tile_pool(name="pers", bufs=1) as pp,
            tc.tile_pool(name="pg", bufs=1, space="PSUM") as pg,
            tc.tile_pool(name="ph", bufs=1, space="PSUM") as ph,
            tc.tile_pool(name="pt", bufs=1, space="PSUM") as pt,
        ):
            # ---- load weights ----
            def w2(name, n):   # [L, n] -> [128, 2, n]
                t_ = wp.tile([128, 2, n], F32, tag=name)
                nc.sync.dma_start(t_[:], d[name][:].rearrange("(k p) n -> p k n", p=128))
                return t_
            def w1(name, p0, n):
                t_ = wp.tile([p0, n], F32, tag=name)
                nc.sync.dma_start(t_[:], d[name][:])
                return t_

            Whp_sb = w2("Wh_plain", NG); Whf_sb = w2("Wh_fb", NG); Mc_sb = w2("Mcolor", NG)
            Mt_sb = w1("Mtype33", 33, NG); b0_sb = w1("bias0", 1, NG)
            Whd_sb = w2("W_heads", NH); bh_sb = w1("b_heads", 1, NH)
            Wh0_sb = w2("Wh0", L); Wc0_sb = w2("Wc0", L)
            bh0_sb = w1("bh0", 1, L); bc0_sb = w1("bc0", 1, L)
            Wl_sb = w2("W_len", T); bl_sb = w1("b_len", 1, T)
            z_sb = w1("z", B, L)

            ident = pp.tile([128, 128], F32, tag="ident")
            masks.make_identity(nc, ident[:])
            ones1 = pp.tile([1, B], F32, tag="ones1")
            nc.vector.memset(ones1[:], 1.0)
            ohtT = pp.tile([33, B], F32, tag="ohtT")   # one-hot type ^T + ones row
            nc.vector.memset(ohtT[32:33, :], 1.0)

            # ---- setup: zT, h0, c0, out_len ----
            ps_zT = pt.tile([128, 2, B], F32, tag="ptA")
            for k in range(2):
                nc.tensor.transpose(ps_zT[:, k, :], z_sb[:, k * 128:(k + 1) * 128],
                                    ident[0:B, 0:B])
            zT_sb = kp.tile([128, 2, B], F32, tag="zT")
            nc.scalar.activation(zT_sb[:], ps_zT[:], AF.Copy)

            h_sb = sp.tile([B, L], F32, tag="h")
            c0_sb = kp.tile([B, L], F32, tag="c0")
            for dst, Wsb, bsb in ((h_sb, Wh0_sb, bh0_sb), (c0_sb, Wc0_sb, bc0_sb)):
                ps = ph.tile([B, 512], F32, tag="phh")
                nc.tensor.matmul(ps[:, 0:L], zT_sb[:, 0, :], Wsb[:, 0, :], start=True, stop=False)
                nc.tensor.matmul(ps[:, 0:L], zT_sb[:, 1, :], Wsb[:, 1, :], start=False, stop=False)
                nc.tensor.matmul(ps[:, 0:L], ones1[:], bsb[:], start=False, stop=True)
                nc.scalar.activation(dst[:], ps[:, 0:L], AF.Tanh)
            s_sb = sp.tile([B, L], F32, tag="c")      # state s = 2*c
            nc.vector.tensor_scalar(s_sb[:], c0_sb[:], 2.0, None, AluOpType.mult)

            ps_len = ph.tile([B, 512], F32, tag="phh")
            nc.tensor.matmul(ps_len[:, 0:T], zT_sb[:, 0, :], Wl_sb[:, 0, :], start=True, stop=False)
            nc.tensor.matmul(ps_len[:, 0:T], zT_sb[:, 1, :], Wl_sb[:, 1, :], start=False, stop=False)
            nc.tensor.matmul(ps_len[:, 0:T], ones1[:], bl_sb[:], start=False, stop=True)
            len_sb = kp.tile([B, T], F32, tag="len")
            nc.scalar.activation(len_sb[:], ps_len[:, 0:T], AF.Copy)
            nc.sync.dma_start(d["out_len"][:], len_sb[:])

            hT_sb = kp.tile([128, 2, B], F32, tag="hT")
            ps_hT = pt.tile([128, 2, B], F32, tag="ptA")
            for k in range(2):
                nc.tensor.transpose(ps_hT[:, k, :], h_sb[:, k * 128:(k + 1) * 128],
                                    ident[0:B, 0:B])
            nc.scalar.activation(hT_sb[:], ps_hT[:], AF.Copy)

            ohsT_sb = None
            # ---- the scan ----
            for t in range(n_steps):
                # gates PSUM [B, 1024]; order (i,f,o,g) after permutation
                psg = pg.tile([B, NG], F32, tag="gates")
                if t == 0:
                    for half in range(2):
                        s = slice(half * 512, half * 512 + 512)
                        nc.tensor.matmul(psg[:, s], hT_sb[:, 0, :], Whp_sb[:, 0, s],
                                         start=True, stop=False)
                        nc.tensor.matmul(psg[:, s], hT_sb[:, 1, :], Whp_sb[:, 1, s],
                                         start=False, stop=False)
                        nc.tensor.matmul(psg[:, s], ones1[:], b0_sb[:, s],
                                         start=False, stop=True)
                else:
                    for half in range(2):
                        s = slice(half * 512, half * 512 + 512)
                        nc.tensor.matmul(psg[:, s], hT_sb[:, 0, :], Whf_sb[:, 0, s],
                                         start=True, stop=False)
                        nc.tensor.matmul(psg[:, s], hT_sb[:, 1, :], Whf_sb[:, 1, s],
                                         start=False, stop=False)
                        nc.tensor.matmul(psg[:, s], ohsT_sb[:, 0, :], Mc_sb[:, 0, s],
                                         start=False, stop=False)
                        nc.tensor.matmul(psg[:, s], ohsT_sb[:, 1, :], Mc_sb[:, 1, s],
                                         start=False, stop=False)
                        nc.tensor.matmul(psg[:, s], ohtT[:], Mt_sb[:, s],
                                         start=False, stop=True)

                # LSTM cell, tanh-only form; i=[0:256] f=[256:512] o=[512:768] g=[768:1024]
                # sigma(x) = 0.5 + 0.5*tanh(x/2); state s = 2c, h2 = 2h (weights absorb the 0.5)
                tifo = kp.tile([B, 768], F32, tag="tifo")     # tanh(i/2)|tanh(f/2)|tanh(o/2)
                tg_sb = kp.tile([B, L], F32, tag="tg")
                nc.scalar.activation(tifo[:], psg[:, 0:768], AF.Tanh, scale=0.5)
                nc.scalar.activation(tg_sb[:], psg[:, 768:NG], AF.Tanh)
                ti, tf, to = tifo[:, 0:256], tifo[:, 256:512], tifo[:, 512:768]
                q1 = kp.tile([B, L], F32, tag="q1")
                nc.vector.scalar_tensor_tensor(q1[:], s_sb[:], 0.5, tf,
                                               AluOpType.mult, AluOpType.mult)  # c*tf
                q2 = kp.tile([B, L], F32, tag="q2")
                nc.gpsimd.tensor_tensor(q2[:], ti, tg_sb[:], AluOpType.mult)     # ti*tg
                r1 = kp.tile([B, L], F32, tag="r1")
                nc.vector.scalar_tensor_tensor(r1[:], s_sb[:], 0.5, q1[:],
                                               AluOpType.mult, AluOpType.add)    # c + c*tf
                r2 = kp.tile([B, L], F32, tag="r2")
                nc.gpsimd.tensor_tensor(r2[:], tg_sb[:], q2[:], AluOpType.add)   # tg + ti*tg
                s_new = sp.tile([B, L], F32, tag="c")
                nc.vector.tensor_tensor(s_new[:], r1[:], r2[:], AluOpType.add)   # = 2c'
                th_sb = kp.tile([B, L], F32, tag="th")
                nc.scalar.activation(th_sb[:], s_new[:], AF.Tanh, scale=0.5)     # tanh(c')
                m_sb = kp.tile([B, L], F32, tag="m")
                nc.gpsimd.tensor_tensor(m_sb[:], to, th_sb[:], AluOpType.mult)   # to*th
                h_new = sp.tile([B, L], F32, tag="h")
                nc.vector.tensor_tensor(h_new[:], th_sb[:], m_sb[:], AluOpType.add)  # = 2h
                s_sb, h_sb = s_new, h_new

                # transpose h
                hT_sb = kp.tile([128, 2, B], F32, tag="hT")
                ps_hT = pt.tile([128, 2, B], F32, tag="ptA")
                for k in range(2):
                    nc.tensor.transpose(ps_hT[:, k, :], h_sb[:, k * 128:(k + 1) * 128],
                                        ident[0:B, 0:B])
                nc.scalar.activation(hT_sb[:], ps_hT[:], AF.Copy)

                # heads: [0:32] type, [32:800] color, [800:804] pos
                psh = ph.tile([B, NH], F32, tag="phh")
                for k in range(2):
                    nc.tensor.matmul(psh[:, 0:512], hT_sb[:, k, :], Whd_sb[:, k, 0:512],
                                     start=(k == 0), stop=False)
                    nc.tensor.matmul(psh[:, 512:NH], hT_sb[:, k, :], Whd_sb[:, k, 512:NH],
                                     start=(k == 0), stop=False)
                nc.tensor.matmul(psh[:, 0:512], ones1[:], bh_sb[:, 0:512],
                                 start=False, stop=True)
                nc.tensor.matmul(psh[:, 512:NH], ones1[:], bh_sb[:, 512:NH],
                                 start=False, stop=True)

                stage = gp.tile([B, NH], F32, tag="stage")
                nc.scalar.activation(stage[:], psh[:], AF.Copy)

                # argmax -> one-hot (color on DVE from PSUM, type on GPSIMD from SBUF)
                maxc = kp.tile([B, 3], F32, tag="maxc")
                nc.vector.tensor_reduce(
                    maxc[:], psh[:, 32:800].rearrange("p (c v) -> p c v", v=256),
                    mybir.AxisListType.X, AluOpType.max)
                oh_sb = kp.tile([B, 3, 256], F32, tag="oh")
                for ch in range(3):
                    nc.vector.tensor_scalar(
                        oh_sb[:, ch, :], psh[:, 32 + 256 * ch: 32 + 256 * (ch + 1)],
                        maxc[:, ch:ch + 1], None, AluOpType.is_equal)
                maxt = kp.tile([B, 1], F32, tag="maxt")
                nc.vector.tensor_reduce(maxt[:], psh[:, 0:32],
                                        mybir.AxisListType.X, AluOpType.max)
                oht_sb = kp.tile([B, 32], F32, tag="oht")
                nc.gpsimd.tensor_scalar(oht_sb[:], stage[:, 0:32], maxt[:], None,
                                        AluOpType.is_equal)

                # sum color one-hots, transpose feedback operands
                ohs_sb = kp.tile([B, 256], F32, tag="ohs")
                nc.vector.tensor_tensor(ohs_sb[:], oh_sb[:, 0, :], oh_sb[:, 1, :], AluOpType.add)
                ohsum_sb = kp.tile([B, 256], F32, tag="ohsum")
                nc.vector.tensor_tensor(ohsum_sb[:], ohs_sb[:], oh_sb[:, 2, :], AluOpType.add)

                ohsT_sb = kp.tile([128, 2, B], F32, tag="ohsT")
                ps_oT = pt.tile([128, 2, B], F32, tag="ptB")
                for k in range(2):
                    nc.tensor.transpose(ps_oT[:, k, :], ohsum_sb[:, k * 128:(k + 1) * 128],
                                        ident[0:B, 0:B])
                nc.scalar.activation(ohsT_sb[:], ps_oT[:], AF.Copy)
                ps_tT = pt.tile([32, B], F32, tag="ptC")
                nc.tensor.transpose(ps_tT[:], oht_sb[:], ident[0:B, 0:B])
                nc.scalar.activation(ohtT[0:32, :], ps_tT[:], AF.Copy)

                # stream outputs
                nc.sync.dma_start(d["out_type"][:, t, :], stage[:, 0:32])
                nc.sync.dma_start(d["out_color"][:, t, :], stage[:, 32:800])
                nc.sync.dma_start(d["out_pos"][:, t, :], stage[:, 800:NH])
    return nc


def kernel(**inputs):
    _patch_tile_drain()
    inputs = {k: np.asarray(v) for k, v in inputs.items()}
    folded = fold_weights(inputs)
    nc = build_nc(T)
    z = np.asarray(inputs["z"], np.float32)
    in_maps = []
    for c in range(N_CORES):
        m = dict(folded)
        m["z"] = np.ascontiguousarray(z[c * B:(c + 1) * B])
        in_maps.append(m)
    res = run_bass_kernel_spmd(nc, in_maps, list(range(N_CORES)))
    BT = 512
    out_length = np.empty((BT, 1, T), np.float32)
    out_type = np.empty((BT, T, 1, 32), np.float32)
    out_pos = np.empty((BT, T, 4), np.float32)
    out_color = np.empty((BT, T, 3, 256), np.float32)
    for c in range(N_CORES):
        r = res.results[c]
        sl = slice(c * B, (c + 1) * B)
        out_length[sl, 0, :] = np.asarray(r["out_len"])
        out_type[sl, :, 0, :] = np.asarray(r["out_type"])
        out_pos[sl] = np.asarray(r["out_pos"])
        out_color[sl] = np.asarray(r["out_color"]).reshape(B, T, 3, 256)
    return out_length, out_type, out_pos, out_color
